# revision 3
# baseline (speedup 1.0000x reference)
"""Trainium2 Bass kernel for nn_AudNet (4-layer LIF SNN, 81-step scan), v3.

Strategy (per core, batch 128 of 1024; data-parallel over 8 cores):
  - Features on partitions, batch on the free dim.
  - Timesteps processed in QUADS (4 steps): every matmul has free dim 512,
    so the fp32r weight-load (LDWEIGHTS ~187ns) fully hides under the
    213ns moving-data stream, and instruction count halves vs pairs.
  - All weights split into fp32r hi + lo terms (residual ~2^-26).
  - PSUM is evacuated to SBUF by the (idle) scalar engine right after
    each m-chunk's accumulation group; the DVE LIF scans read cur from
    SBUF.  This frees PSUM banks and decouples PE from DVE.
  - Software pipeline with a one-quad lag for layers 3/4:
        PE stream:  mm2(q) | mm1(q+1) | mm3(q-1) mm4(q-1) rec(q-1)
        DVE stream: lif2(q) | lif3(q-1) lif4(q-1) | lif1(q+1)
    so every PE instruction's inputs are ready long before the PE
    reaches it -> the PE never idles -> the HAM clock gate stays at
    8/8 (2.4 GHz) instead of oscillating to 4/8.
  - Biases ride inside the matmuls via constant-one rows in padding
    partitions of the stationary tiles.
  - LIF per step: mem = b*mem + cur (STT on DVE, cur from SBUF);
    mem -= spk_prev (tensor_sub on the otherwise-idle GpSimd engine --
    the sustained PE+DVE power draw trips a ~50%-duty HAM clock-gate,
    so DVE duty is kept as low as possible); spk = mem > 1 (GT).
  - spk1/spk2 are 8-slot rings (slot = t mod 8) so lif1(q+1) writes
    the opposite half from the slots mm2(q) is reading.  Slot-major
    layout keeps the DVE/GpSimd accesses unit-stride (the matmul pays
    a small strided-read penalty instead; the DVE side is the one
    coupled to the power cap).
  - x is transposed to [F, T*B] on the host so the per-quad input
    loads are contiguous block DMAs.
  - Outputs are PE-transposed into a PSUM block accumulator, bounced
    to SBUF by the scalar engine every 8 steps, and DMA'd out per
    block.
"""

import numpy as np

import concourse.bass as bass
import concourse.mybir as mybir
import concourse.tile as tile
from concourse.bass_utils import run_bass_kernel_spmd

F32 = mybir.dt.float32
F32R = mybir.dt.float32r
GT = mybir.AluOpType.is_gt
MULT = mybir.AluOpType.mult
ADD = mybir.AluOpType.add

T_FULL = 81
F = 129          # input features per step
H = 1000         # hidden width
HP = 1024        # padded hidden width (8 chunks of 128)
HL = 20          # layer-3 width
O = 10           # output width
B = 128          # batch per core
NCORES = 8
BETA = 0.95
TH = 1.0
Q = 4            # steps per quad
NSLOT = 8        # spk1/spk2 ring slots


def fix_multi_waits(nc, limit=1):
    """walrus codegen rejects >1 sem wait on most instructions; spill
    extras onto standalone EventSemaphore instructions in front."""
    ev = 0
    for bb in nc.main_func.blocks:
        out = []
        for ins in bb.instructions:
            si = ins.sync_info
            if si is not None and len(si.on_wait) > limit:
                waits = list(si.on_wait)
                extra, keep = waits[:-limit], waits[-limit:]
                for w in extra:
                    e = mybir.InstEventSemaphore(name=f"evw_{ev}", ins=[], outs=[])
                    ev += 1
                    e.engine = ins.engine
                    e.sync_info = mybir.SyncInfo(on_wait=[w], on_update=[])
                    out.append(e)
                ins.sync_info = mybir.SyncInfo(on_wait=keep, on_update=list(si.on_update))
            out.append(ins)
        bb.instructions = out


def build_nc(T=T_FULL):
    NQ = (T + Q - 1) // Q  # 21 quads; the last one has T - Q*(NQ-1) real steps

    nc = bass.Bass()

    def rd(ap):
        """view a reduced-dtype AP as f32 for DVE arithmetic"""
        return ap.bitcast(F32)

    def rr(ap):
        """view an f32 AP as fp32r (bit-identical storage)"""
        return ap.bitcast(F32R)

    x_d = nc.declare_dram_parameter("xt", [F, T_FULL * B], F32, isOutput=False)
    W1_d = nc.declare_dram_parameter("W1", [H, F], F32, isOutput=False)
    b1_d = nc.declare_dram_parameter("b1", [H], F32, isOutput=False)
    W2_d = nc.declare_dram_parameter("W2", [H, H], F32, isOutput=False)
    b2_d = nc.declare_dram_parameter("b2", [H], F32, isOutput=False)
    W3_d = nc.declare_dram_parameter("W3", [HL, H], F32, isOutput=False)
    b3_d = nc.declare_dram_parameter("b3", [HL], F32, isOutput=False)
    W4_d = nc.declare_dram_parameter("W4", [O, HL], F32, isOutput=False)
    b4_d = nc.declare_dram_parameter("b4", [O], F32, isOutput=False)
    eye_d = nc.declare_dram_parameter("eye", [128, 128], F32, isOutput=False)
    spk_o = nc.declare_dram_parameter("spk_out", [T, B, O], F32, isOutput=True)
    mem_o = nc.declare_dram_parameter("mem_out", [T, B, O], F32, isOutput=True)

    with tile.TileContext(nc) as tc:
        with tc.tile_pool(name="persist", bufs=1) as pp:
            # ---- persistent SBUF tiles ----
            eye = pp.tile([128, 128], F32, tag="eye")
            w1h = pp.tile([128, HP], F32R, tag="w1h")
            w1l = pp.tile([128, HP], F32R, tag="w1l")
            # stacked L1 bias stationary: rows 0-1 = hi {feat128, bias},
            # rows 32-33 = lo (32-aligned for DVE writes); rows 2-31 = 0 so
            # the junk rows of the stacked moving operand contribute nothing.
            w1b4 = pp.tile([34, HP], F32R, tag="w1b4")
            w2h = [pp.tile([128, HP], F32R, tag=f"w2h{c}", name=f"w2h{c}") for c in range(8)]
            w2l = [pp.tile([128, HP], F32R, tag=f"w2l{c}", name=f"w2l{c}") for c in range(8)]
            w3h = pp.tile([128, 8 * HL], F32R, tag="w3h")
            w3l = pp.tile([128, 8 * HL], F32R, tag="w3l")
            w4h = pp.tile([HL + 1, O], F32R, tag="w4h")
            w4l = pp.tile([HL + 1, O], F32R, tag="w4l")
            mem1 = pp.tile([128, HP], F32, tag="mem1")
            mem2 = pp.tile([128, HP], F32, tag="mem2")
            mem3 = pp.tile([HL, B], F32, tag="mem3")
            # chunk-major spike rings: [c(8), slot(8), b] so the mm2/mm3
            # moving operands (4 consecutive slots of one chunk) are
            # contiguous 512-element runs.
            spk1 = pp.tile([128, NSLOT * HP], F32R, tag="spk1")   # slot = t mod 8
            spk2 = pp.tile([128, NSLOT * HP], F32R, tag="spk2")
            spk3 = pp.tile([HL + 1, Q * B], F32R, tag="spk3")     # row HL = ones
            spk4 = pp.tile([O, Q * B], F32, tag="spk4")
            # mem4 rows 0-9; row 10 = f32 ones (DMA source for xbr)
            mem4x = pp.tile([11, Q * B], F32, tag="mem4x")
            cur1 = pp.tile([128, 8 * Q * B], F32, tag="cur1")     # [c(8), s(4), b]
            cur2 = pp.tile([128, 8 * Q * B], F32, tag="cur2")
            obuf = pp.tile([128, 8 * 2 * O], F32, tag="obuf")     # out bounce

            # setup-time scratch carved out of cur1/cur2 (they are only
            # written by the scan).  b1*, b3*, b4* in cur1; b2*, ones in
            # cur2 (all on partition 0; disjoint column ranges).
            b1s, b1h, b1l = cur1[0:1, 0:H], rr(cur1[0:1, 1024:1024+H]), rr(cur1[0:1, 2048:2048+H])
            b3s, b3h, b3l = cur1[0:1, 3072:3072+HL], rr(cur1[0:1, 3104:3104+HL]), rr(cur1[0:1, 3136:3136+HL])
            b4s, b4h, b4l = cur1[0:1, 3200:3200+O], rr(cur1[0:1, 3232:3232+O]), rr(cur1[0:1, 3264:3264+O])
            b2s, b2h, b2l = cur2[0:1, 0:H], rr(cur2[0:1, 1024:1024+H]), rr(cur2[0:1, 2048:2048+H])
            ones_r = rr(cur2[0:1, 3072:3072+NSLOT*B])             # fp32r ones

            nc.sync.dma_start(out=eye[:], in_=eye_d[:])

            # ================= SETUP =================
            with (
                tc.tile_pool(name="setup_sb", bufs=1) as sp,
                tc.tile_pool(name="setup_ps", bufs=4, space="PSUM") as spp,
            ):
                # bias splits (hi/lo in fp32r)
                def bias_split(b_dram, n, bs, bh, bl):
                    nc.sync.dma_start(out=bs, in_=b_dram[:].rearrange("(a n) -> a n", a=1))
                    nc.vector.tensor_copy(out=bh, in_=bs)
                    nc.vector.tensor_sub(bl, bs, rd(bh))

                bias_split(b1_d, H, b1s, b1h, b1l)
                bias_split(b2_d, H, b2s, b2h, b2l)
                bias_split(b3_d, HL, b3s, b3h, b3l)
                bias_split(b4_d, O, b4s, b4h, b4l)

                # zero-init all weight tiles (padding regions stay 0)
                for tl in [w1h, w1l, w1b4, w3h, w3l, w4h, w4l] + w2h + w2l:
                    nc.vector.memset(rd(tl[:]), 0.0)

                def evac_split(psum_ap, hi_ap, lo_ap):
                    nc.vector.tensor_copy(out=hi_ap, in_=psum_ap)
                    nc.vector.tensor_sub(lo_ap, psum_ap, rd(hi_ap))

                # prefetch the small weight sources early
                w3s = sp.tile([HL, H], F32, tag="w3s")
                nc.sync.dma_start(out=w3s[:], in_=W3_d[:])
                w4s = sp.tile([O, HL], F32, tag="w4s")
                nc.sync.dma_start(out=w4s[:], in_=W4_d[:])

                # ---- W2 ----
                for mc in range(8):
                    mh = 128 if mc < 7 else H - 7 * 128
                    ws = sp.tile([128, H], F32, tag="w2s", bufs=2)
                    nc.sync.dma_start(out=ws[:mh, :], in_=W2_d[mc * 128 : mc * 128 + mh, :])
                    for c in range(8):
                        kw = 128 if c < 7 else H - 7 * 128
                        pt = spp.tile([128, 128], F32, tag="tp")
                        nc.tensor.transpose(
                            pt[:kw, :mh], ws[:mh, c * 128 : c * 128 + kw], eye[:mh, :mh]
                        )
                        evac_split(
                            pt[:kw, :mh],
                            w2h[c][:kw, mc * 128 : mc * 128 + mh],
                            w2l[c][:kw, mc * 128 : mc * 128 + mh],
                        )
                # bias rows: k-chunk 7, partition 104 (feature 1000)
                nc.sync.dma_start(out=w2h[7][104:105, 0:H], in_=b2h)
                nc.sync.dma_start(out=w2l[7][104:105, 0:H], in_=b2l)

                # ---- W1 ----
                for mc in range(8):
                    mh = 128 if mc < 7 else H - 7 * 128
                    ws = sp.tile([128, F], F32, tag="w1s", bufs=2)
                    nc.sync.dma_start(out=ws[:mh, :], in_=W1_d[mc * 128 : mc * 128 + mh, :])
                    pt = spp.tile([128, 128], F32, tag="tp")
                    nc.tensor.transpose(pt[:128, :mh], ws[:mh, 0:128], eye[:mh, :mh])
                    evac_split(
                        pt[:128, :mh],
                        w1h[:, mc * 128 : mc * 128 + mh],
                        w1l[:, mc * 128 : mc * 128 + mh],
                    )
                    pt2 = spp.tile([128, 128], F32, tag="tp2")
                    nc.tensor.transpose(pt2[:1, :mh], ws[:mh, 128:129], eye[:mh, :mh])
                    evac_split(
                        pt2[:1, :mh],
                        w1b4[0:1, mc * 128 : mc * 128 + mh],
                        w1b4[32:33, mc * 128 : mc * 128 + mh],
                    )
                nc.sync.dma_start(out=w1b4[1:2, 0:H], in_=b1h)
                nc.sync.dma_start(out=w1b4[33:34, 0:H], in_=b1l)

                # ---- W3 ----
                for c in range(8):
                    kw = 128 if c < 7 else H - 7 * 128
                    pt = spp.tile([128, 128], F32, tag="tp")
                    nc.tensor.transpose(
                        pt[:kw, :HL], w3s[:, c * 128 : c * 128 + kw], eye[:HL, :HL]
                    )
                    evac_split(
                        pt[:kw, :HL],
                        w3h[:kw, c * HL : (c + 1) * HL],
                        w3l[:kw, c * HL : (c + 1) * HL],
                    )
                nc.sync.dma_start(out=w3h[104:105, 7 * HL : 8 * HL], in_=b3h)
                nc.sync.dma_start(out=w3l[104:105, 7 * HL : 8 * HL], in_=b3l)

                # ---- W4 ----
                pt = spp.tile([128, 128], F32, tag="tp")
                nc.tensor.transpose(pt[:HL, :O], w4s[:, :], eye[:O, :O])
                evac_split(pt[:HL, :O], w4h[:HL, :], w4l[:HL, :])
                nc.sync.dma_start(out=w4h[HL : HL + 1, :], in_=b4h)
                nc.sync.dma_start(out=w4l[HL : HL + 1, :], in_=b4l)

                # ---- state init ----
                for tl in [mem1, mem2, mem3]:
                    nc.vector.memset(tl[:], 0.0)
                nc.vector.memset(spk4[:], 0.0)
                nc.vector.memset(mem4x[0:10, :], 0.0)
                nc.vector.memset(rd(spk1[:]), 0.0)
                nc.vector.memset(rd(spk2[:]), 0.0)
                nc.vector.memset(rd(spk3[:]), 0.0)
                # constant-one bias rhs rows (all ring slots).  Compute
                # engines need 32-aligned partition starts, so write these
                # single rows via DMA from the fp32r ones region.
                nc.vector.memset(rd(ones_r), 1.0)
                s1w = spk1[:].rearrange("p (s c b) -> p s c b", s=NSLOT, b=B)
                s2w = spk2[:].rearrange("p (s c b) -> p s c b", s=NSLOT, b=B)
                ones_v = ones_r.rearrange("p (s b) -> p s b", s=NSLOT)
                nc.sync.dma_start(out=s1w[104:105, :, 7, :], in_=ones_v)
                nc.sync.dma_start(out=s2w[104:105, :, 7, :], in_=ones_v)
                # forever-spike driver for padding neurons
                big_r = rr(cur1[0:1, 3328:3328+24])
                nc.vector.memset(rd(big_r), 64.0)
                nc.sync.dma_start(out=w1b4[1:2, H:HP], in_=big_r)
                nc.sync.dma_start(out=w2h[7][104:105, H:HP], in_=big_r)
                nc.sync.dma_start(
                    out=spk3[HL : HL + 1, :],
                    in_=ones_r.rearrange("p (s b) -> p s b", s=2)[:, 0, :],
                )
                # f32 ones row (1.0 has identical f32r/f32 bits)
                nc.sync.dma_start(
                    out=mem4x[10:11, :],
                    in_=rd(ones_r).rearrange("p (s b) -> p s b", s=2)[:, 0, :],
                )
                # clear the scratch regions before the scan
                nc.vector.memset(cur1[:], 0.0)
                nc.vector.memset(cur2[:], 0.0)

            # ================= SCAN =================
            with (
                tc.tile_pool(name="xdma", bufs=2) as xdp,
                tc.tile_pool(name="xbdma", bufs=1) as xbp,
                tc.tile_pool(name="xsplit", bufs=1) as xsp,
                tc.tile_pool(name="pl1", bufs=3, space="PSUM") as pl1,
                tc.tile_pool(name="pl2", bufs=2, space="PSUM") as pl2,
                tc.tile_pool(name="pl34", bufs=1, space="PSUM") as pl34,
                tc.tile_pool(name="pout", bufs=1, space="PSUM") as pout,
            ):
                evac = {"done": 0, "tile": None}

                xv = x_d[:].rearrange("f (t b) -> f t b", b=B)
                s1_v = spk1[:].rearrange("p (s c b) -> p s c b", s=NSLOT, b=B)
                s2_v = spk2[:].rearrange("p (s c b) -> p s c b", s=NSLOT, b=B)
                m1_v = mem1[:].rearrange("p (c b) -> p c b", b=B)
                m2_v = mem2[:].rearrange("p (c b) -> p c b", b=B)
                c1_v = cur1[:].rearrange("p (c s b) -> p c s b", c=8, b=B)
                c2_v = cur2[:].rearrange("p (c s b) -> p c s b", c=8, b=B)
                s3_v = spk3[:].rearrange("p (s b) -> p s b", s=Q)
                mem4 = mem4x[0:O, :]
                s4_v = spk4[:].rearrange("p (s b) -> p s b", s=Q)
                m4_v = mem4.rearrange("p (s b) -> p s b", s=Q)

                def split_x(q):
                    """DMA the x slice for quad q and make fp32r hi/lo
                    splits (exact for x).  Steps past T keep stale (finite)
                    ring data; they are never read back meaningfully."""
                    t0 = Q * q
                    nt = min(Q, T - t0)
                    xq = xdp.tile([128, Q * B], F32, tag="xq")
                    xbr = xbp.tile([2, Q * B], F32, tag="xbr")
                    xq_w = xq[:].rearrange("p (t b) -> p t b", b=B)
                    xbr_w = xbr[:].rearrange("p (t b) -> p t b", b=B)
                    nc.sync.dma_start(
                        out=xq_w[:, 0:nt, :], in_=xv[0:128, t0 : t0 + nt, :]
                    )
                    nc.sync.dma_start(
                        out=xbr_w[0:1, 0:nt, :], in_=xv[128:129, t0 : t0 + nt, :]
                    )
                    if q < 1:
                        # single buffer: write the ones row once
                        nc.sync.dma_start(out=xbr[1:2, :], in_=mem4x[10:11, :])
                    xh = xsp.tile([128, Q * B], F32R, tag="xh")
                    xl = xsp.tile([128, Q * B], F32R, tag="xl")
                    xbl = xsp.tile([2, Q * B], F32R, tag="xbl")
                    # stacked moving operand for the L1 bias terms: rows 0-1
                    # and 32-33 both carry {feat128-hi, ones}; rows 2-31 stay
                    # zero from the pre-loop memset (bufs=1: same memory).
                    xb34 = xsp.tile([34, Q * B], F32R, tag="xb34")
                    nc.scalar.copy(out=xh[:], in_=xq[:])
                    nc.vector.tensor_sub(xl[:], xq[:], rd(xh[:]))
                    nc.scalar.copy(out=xb34[0:2, :], in_=xbr[:])
                    nc.scalar.copy(out=xb34[32:34, :], in_=xbr[:])
                    nc.vector.tensor_sub(xbl[:], xbr[:], rd(xb34[0:2, :]))
                    return xh, xl, xbl, xb34

                def mm1(q, xs):
                    """L1 matmuls for quad q; evac each m-chunk to cur1."""
                    xh, xl, xbl, xb34 = xs
                    for mc in range(8):
                        t1 = pl1.tile([128, Q * B], F32, tag="l1")
                        ms = slice(mc * 128, (mc + 1) * 128)
                        terms = [
                            (w1h[:, ms], xh[:]),
                            (w1b4[:, ms], xb34[:]),
                            (w1h[:, ms], xl[:]),
                            (w1b4[0:2, ms], xbl[:]),
                            (w1l[:, ms], xh[:]),
                        ]
                        for i, (lhsT, rhs) in enumerate(terms):
                            nc.tensor.matmul(
                                t1[:], lhsT=lhsT, rhs=rhs,
                                start=(i == 0), stop=(i == len(terms) - 1),
                            )
                        nc.scalar.copy(out=c1_v[:, mc, :, :], in_=t1[:])

                def mm2(q):
                    """L2 matmuls for quad q; evac each m-chunk to cur2."""
                    sl0 = (Q * q) % NSLOT
                    for mc in range(8):
                        t2 = pl2.tile([128, Q * B], F32, tag="l2")
                        n = 0
                        for wsp in (w2h, w2l):
                            for c in range(8):
                                nc.tensor.matmul(
                                    t2[:],
                                    lhsT=wsp[c][:, mc * 128 : (mc + 1) * 128],
                                    rhs=s1_v[:, sl0 : sl0 + Q, c, :],
                                    start=(n == 0), stop=(n == 15),
                                )
                                n += 1
                        nc.scalar.copy(out=c2_v[:, mc, :, :], in_=t2[:])

                def lif12(q, m_v, c_v, s_v, mem, spk, eng):
                    t0 = Q * q
                    for s in range(Q):
                        t = t0 + s
                        if t >= T:
                            break
                        sl, slp = t % NSLOT, (t - 1) % NSLOT
                        eng.scalar_tensor_tensor(
                            out=m_v[:, :, :], in0=m_v[:, :, :], scalar=BETA,
                            in1=c_v[:, :, s, :], op0=MULT, op1=ADD,
                        )
                        nc.gpsimd.tensor_sub(
                            mem[:], mem[:],
                            rd(spk[:, slp * HP : (slp + 1) * HP]),
                        )
                        eng.tensor_scalar(
                            out=s_v[:, sl, 0:7, :], in0=m_v[:, 0:7, :],
                            scalar1=TH, scalar2=None, op0=GT,
                        )
                        eng.tensor_scalar(
                            out=s_v[0:104, sl, 7, :], in0=m_v[0:104, 7, :],
                            scalar1=TH, scalar2=None, op0=GT,
                        )

                def mm3(q):
                    sl0 = (Q * q) % NSLOT
                    t3 = pl34.tile([32, Q * B], F32, tag="l3")
                    n = 0
                    for wsp in (w3h, w3l):
                        for c in range(8):
                            nc.tensor.matmul(
                                t3[0:HL, :],
                                lhsT=wsp[:, c * HL : (c + 1) * HL],
                                rhs=s2_v[:, sl0 : sl0 + Q, c, :],
                                start=(n == 0), stop=(n == 15),
                            )
                            n += 1
                    return t3

                def lif3(q, t3):
                    t0 = Q * q
                    for s in range(Q):
                        t = t0 + s
                        if t >= T:
                            break
                        sl, slp = t % Q, (t - 1) % Q
                        nc.vector.scalar_tensor_tensor(
                            out=mem3[:], in0=mem3[:], scalar=BETA,
                            in1=t3[0:HL, s * B : (s + 1) * B], op0=MULT, op1=ADD,
                        )
                        nc.gpsimd.tensor_sub(
                            mem3[:], mem3[:], rd(s3_v[0:HL, slp, :])
                        )
                        nc.vector.tensor_scalar(
                            out=s3_v[0:HL, sl, :], in0=mem3[:],
                            scalar1=TH, scalar2=None, op0=GT,
                        )

                def mm4(q):
                    t4 = pl34.tile([32, Q * B], F32, tag="l4")
                    nc.tensor.matmul(
                        t4[0:O, :], lhsT=w4h[:], rhs=spk3[:],
                        start=True, stop=False,
                    )
                    nc.tensor.matmul(
                        t4[0:O, :], lhsT=w4l[:], rhs=spk3[:],
                        start=False, stop=True,
                    )
                    return t4

                def lif4(q, t4):
                    t0 = Q * q
                    for s in range(Q):
                        t = t0 + s
                        if t >= T:
                            break
                        sl, slp = t % Q, (t - 1) % Q
                        nc.vector.scalar_tensor_tensor(
                            out=m4_v[:, sl, :], in0=m4_v[:, slp, :], scalar=BETA,
                            in1=t4[0:O, s * B : (s + 1) * B], op0=MULT, op1=ADD,
                        )
                        nc.gpsimd.tensor_sub(
                            m4_v[:, sl, :], m4_v[:, sl, :], s4_v[:, slp, :]
                        )
                        nc.vector.tensor_scalar(
                            out=s4_v[:, sl, :], in0=m4_v[:, sl, :],
                            scalar1=TH, scalar2=None, op0=GT,
                        )

                def flush_block(t_end):
                    """bounce the finished 12-step block PSUM->SBUF on the
                    scalar engine, then DMA it to DRAM."""
                    d0 = evac["done"]
                    n = t_end - d0
                    blk = evac["tile"]
                    nc.scalar.copy(out=obuf[:, 0 : n * 2 * O], in_=blk[:, 0 : n * 2 * O])
                    ob = obuf[:, 0 : n * 2 * O].rearrange("b (t x) -> b t x", x=2 * O)
                    nc.sync.dma_start(
                        out=spk_o[d0:t_end].rearrange("t b o -> b t o"),
                        in_=ob[:, :, 0:O],
                    )
                    nc.sync.dma_start(
                        out=mem_o[d0:t_end].rearrange("t b o -> b t o"),
                        in_=ob[:, :, O : 2 * O],
                    )
                    evac["done"] = t_end
                    evac["tile"] = None

                def record(q):
                    t0 = Q * q
                    if evac["tile"] is None:
                        evac["tile"] = pout.tile(
                            [128, 8 * 2 * O], F32, tag="outacc", name="outacc"
                        )
                    outacc = evac["tile"]
                    for s in range(Q):
                        t = t0 + s
                        if t >= T:
                            break
                        sl = t % Q
                        w = t - evac["done"]
                        nc.tensor.transpose(
                            outacc[:, w * 2 * O : w * 2 * O + O],
                            spk4[:, sl * B : (sl + 1) * B],
                            eye[:O, :O],
                        )
                        nc.tensor.transpose(
                            outacc[:, w * 2 * O + O : (w + 1) * 2 * O],
                            mem4[:, sl * B : (sl + 1) * B],
                            eye[:O, :O],
                        )
                    t_end = min(t0 + Q, T)
                    if t_end - evac["done"] == 8 or t_end == T:
                        flush_block(t_end)

                # ---- prologue: zero the stacked-xb pool slot once, then
                # x splits for quads 0/1, L1+lif1(0) ----
                xb34_init = xsp.tile([34, Q * B], F32R, tag="xb34", name="xb34_init")
                nc.vector.memset(rd(xb34_init[:]), 0.0)
                xs = {0: split_x(0), 1: split_x(1)}
                mm1(0, xs[0])
                lif12(0, m1_v, c1_v, s1_v, mem1, spk1, nc.vector)

                # ---- main loop: PE never waits ----
                # PE:   mm1(q+1) | mm2(q) | mm3(q-1) mm4(q-1) rec(q-1)
                # DVE:  lif1(q+1) | x-lo sub(q+2) | lif2(q)
                # Pool: lif3(q-1) lif4(q-1)
                for q in range(NQ):
                    if q + 1 < NQ:
                        mm1(q + 1, xs.pop(q + 1))
                    mm2(q)
                    if q + 1 < NQ:
                        lif12(q + 1, m1_v, c1_v, s1_v, mem1, spk1, nc.vector)
                    if q + 2 < NQ:
                        xs[q + 2] = split_x(q + 2)
                    if q > 0:
                        t3p = mm3(q - 1)
                        lif3(q - 1, t3p)
                        t4p = mm4(q - 1)
                        lif4(q - 1, t4p)
                        record(q - 1)
                    lif12(q, m2_v, c2_v, s2_v, mem2, spk2, nc.vector)

                # ---- epilogue: L3/L4 for the last quad ----
                t3p = mm3(NQ - 1)
                lif3(NQ - 1, t3p)
                t4p = mm4(NQ - 1)
                lif4(NQ - 1, t4p)
                record(NQ - 1)

    fix_multi_waits(nc)
    return nc


_NC_CACHE = {}


def _get_nc(T=T_FULL):
    if T not in _NC_CACHE:
        _NC_CACHE[T] = build_nc(T)
    return _NC_CACHE[T]


def run_cores(inputs, T=T_FULL, n_cores=NCORES, **kw):
    """Run on the first n_cores with batch n_cores*128; returns (spk, mem)."""
    nc = _get_nc(T)
    eye = np.eye(128, dtype=np.float32)
    base = {k: np.asarray(inputs[k], np.float32)
            for k in ("W1", "b1", "W2", "b2", "W3", "b3", "W4", "b4")}
    base["eye"] = eye
    x = np.asarray(inputs["x"], np.float32)
    in_maps = []
    for c in range(n_cores):
        xc = x[c * B : (c + 1) * B].reshape(B, F, T_FULL)
        xt = np.ascontiguousarray(xc.transpose(1, 2, 0)).reshape(F, T_FULL * B)
        in_maps.append(dict(base, xt=xt))
    res = run_bass_kernel_spmd(nc, in_maps, list(range(n_cores)), **kw)
    run_cores.last_result = res
    spk = np.concatenate([res.results[c]["spk_out"] for c in range(n_cores)], axis=1)
    mem = np.concatenate([res.results[c]["mem_out"] for c in range(n_cores)], axis=1)
    return spk, mem


def kernel(x, W1, b1, W2, b2, W3, b3, W4, b4):
    spk, mem = run_cores(
        dict(x=x, W1=W1, b1=b1, W2=W2, b2=b2, W3=W3, b3=b3, W4=W4, b4=b4)
    )
    return spk, mem


# revision 5
# speedup vs baseline: 1.0267x; 1.0267x over previous
"""Trainium2 Bass kernel for nn_AudNet (4-layer LIF SNN, 81-step scan), v3.

Strategy (per core, batch 128 of 1024; data-parallel over 8 cores):
  - Features on partitions, batch on the free dim.
  - Timesteps processed in QUADS (4 steps): every matmul has free dim 512,
    so the fp32r weight-load (LDWEIGHTS ~187ns) fully hides under the
    213ns moving-data stream, and instruction count halves vs pairs.
  - All weights split into fp32r hi + lo terms (residual ~2^-26).
  - PSUM is evacuated to SBUF by the (idle) scalar engine right after
    each m-chunk's accumulation group; the DVE LIF scans read cur from
    SBUF.  This frees PSUM banks and decouples PE from DVE.
  - Software pipeline with a one-quad lag for layers 3/4:
        PE stream:  mm2(q) | mm1(q+1) | mm3(q-1) mm4(q-1) rec(q-1)
        DVE stream: lif2(q) | lif3(q-1) lif4(q-1) | lif1(q+1)
    so every PE instruction's inputs are ready long before the PE
    reaches it -> the PE never idles -> the HAM clock gate stays at
    8/8 (2.4 GHz) instead of oscillating to 4/8.
  - Biases ride inside the matmuls via constant-one rows in padding
    partitions of the stationary tiles.
  - LIF per step: mem = b*mem + cur (STT on DVE, cur from SBUF);
    mem -= spk_prev (tensor_sub on the otherwise-idle GpSimd engine --
    the sustained PE+DVE power draw trips a ~50%-duty HAM clock-gate,
    so DVE duty is kept as low as possible); spk = mem > 1 (GT).
  - spk1/spk2 are 8-slot rings (slot = t mod 8) so lif1(q+1) writes
    the opposite half from the slots mm2(q) is reading.  Slot-major
    layout keeps the DVE/GpSimd accesses unit-stride (the matmul pays
    a small strided-read penalty instead; the DVE side is the one
    coupled to the power cap).
  - x is transposed to [F, T*B] on the host so the per-quad input
    loads are contiguous block DMAs.
  - Outputs are PE-transposed into a PSUM block accumulator, bounced
    to SBUF by the scalar engine every 8 steps, and DMA'd out per
    block.
"""

import numpy as np

import concourse.bass as bass
import concourse.mybir as mybir
import concourse.tile as tile
from concourse.bass_utils import run_bass_kernel_spmd

F32 = mybir.dt.float32
F32R = mybir.dt.float32r
GT = mybir.AluOpType.is_gt
MULT = mybir.AluOpType.mult
ADD = mybir.AluOpType.add

T_FULL = 81
F = 129          # input features per step
H = 1000         # hidden width
HP = 1024        # padded hidden width (8 chunks of 128)
HL = 20          # layer-3 width
O = 10           # output width
B = 128          # batch per core
NCORES = 8
BETA = 0.95
TH = 1.0
Q = 4            # steps per quad
NSLOT = 8        # spk1/spk2 ring slots


def fix_multi_waits(nc, limit=1):
    """walrus codegen rejects >1 sem wait on most instructions; spill
    extras onto standalone EventSemaphore instructions in front."""
    ev = 0
    for bb in nc.main_func.blocks:
        out = []
        for ins in bb.instructions:
            si = ins.sync_info
            if si is not None and len(si.on_wait) > limit:
                waits = list(si.on_wait)
                extra, keep = waits[:-limit], waits[-limit:]
                for w in extra:
                    e = mybir.InstEventSemaphore(name=f"evw_{ev}", ins=[], outs=[])
                    ev += 1
                    e.engine = ins.engine
                    e.sync_info = mybir.SyncInfo(on_wait=[w], on_update=[])
                    out.append(e)
                ins.sync_info = mybir.SyncInfo(on_wait=keep, on_update=list(si.on_update))
            out.append(ins)
        bb.instructions = out


def build_nc(T=T_FULL):
    NQ = (T + Q - 1) // Q  # 21 quads; the last one has T - Q*(NQ-1) real steps

    nc = bass.Bass()

    def rd(ap):
        """view a reduced-dtype AP as f32 for DVE arithmetic"""
        return ap.bitcast(F32)

    def rr(ap):
        """view an f32 AP as fp32r (bit-identical storage)"""
        return ap.bitcast(F32R)

    x_d = nc.declare_dram_parameter("xt", [F, T_FULL * B], F32, isOutput=False)
    W1_d = nc.declare_dram_parameter("w1t", [F, H], F32, isOutput=False)
    b1_d = nc.declare_dram_parameter("b1", [H], F32, isOutput=False)
    W2_d = nc.declare_dram_parameter("w2t", [H, H], F32, isOutput=False)
    b2_d = nc.declare_dram_parameter("b2", [H], F32, isOutput=False)
    W3_d = nc.declare_dram_parameter("w3t", [H, HL], F32, isOutput=False)
    b3_d = nc.declare_dram_parameter("b3", [HL], F32, isOutput=False)
    W4_d = nc.declare_dram_parameter("w4t", [HL, O], F32, isOutput=False)
    b4_d = nc.declare_dram_parameter("b4", [O], F32, isOutput=False)
    eye_d = nc.declare_dram_parameter("eye", [128, 128], F32, isOutput=False)
    spk_o = nc.declare_dram_parameter("spk_out", [T, B, O], F32, isOutput=True)
    mem_o = nc.declare_dram_parameter("mem_out", [T, B, O], F32, isOutput=True)

    with tile.TileContext(nc) as tc:
        with tc.tile_pool(name="persist", bufs=1) as pp:
            # ---- persistent SBUF tiles ----
            eye = pp.tile([128, 128], F32, tag="eye")
            w1h = pp.tile([128, HP], F32R, tag="w1h")
            w1l = pp.tile([128, HP], F32R, tag="w1l")
            # stacked L1 bias stationary: rows 0-1 = hi {feat128, bias},
            # rows 32-33 = lo (32-aligned for DVE writes); rows 2-31 = 0 so
            # the junk rows of the stacked moving operand contribute nothing.
            w1b4 = pp.tile([34, HP], F32R, tag="w1b4")
            w2h = [pp.tile([128, HP], F32R, tag=f"w2h{c}", name=f"w2h{c}") for c in range(8)]
            w2l = [pp.tile([128, HP], F32R, tag=f"w2l{c}", name=f"w2l{c}") for c in range(8)]
            w3h = pp.tile([128, 8 * HL], F32R, tag="w3h")
            w3l = pp.tile([128, 8 * HL], F32R, tag="w3l")
            w4h = pp.tile([HL + 1, O], F32R, tag="w4h")
            w4l = pp.tile([HL + 1, O], F32R, tag="w4l")
            mem1 = pp.tile([128, HP], F32, tag="mem1")
            mem2 = pp.tile([128, HP], F32, tag="mem2")
            mem3 = pp.tile([HL, B], F32, tag="mem3")
            # chunk-major spike rings: [c(8), slot(8), b] so the mm2/mm3
            # moving operands (4 consecutive slots of one chunk) are
            # contiguous 512-element runs.
            spk1 = pp.tile([128, NSLOT * HP], F32R, tag="spk1")   # slot = t mod 8
            spk2 = pp.tile([128, NSLOT * HP], F32R, tag="spk2")
            spk3 = pp.tile([HL + 1, Q * B], F32R, tag="spk3")     # row HL = ones
            spk4 = pp.tile([O, Q * B], F32, tag="spk4")
            # mem4 rows 0-9; row 10 = f32 ones (DMA source for xbr)
            mem4x = pp.tile([11, Q * B], F32, tag="mem4x")
            cur1 = pp.tile([128, 8 * Q * B], F32, tag="cur1")     # [c(8), s(4), b]
            cur2 = pp.tile([128, 8 * Q * B], F32, tag="cur2")
            obuf = pp.tile([128, 8 * 2 * O], F32, tag="obuf")     # out bounce

            # setup-time scratch carved out of cur1/cur2 (they are only
            # written by the scan).  b1*, b3*, b4* in cur1; b2*, ones in
            # cur2 (all on partition 0; disjoint column ranges).
            b1s, b1h, b1l = cur1[0:1, 0:H], rr(cur1[0:1, 1024:1024+H]), rr(cur1[0:1, 2048:2048+H])
            b3s, b3h, b3l = cur1[0:1, 3072:3072+HL], rr(cur1[0:1, 3104:3104+HL]), rr(cur1[0:1, 3136:3136+HL])
            b4s, b4h, b4l = cur1[0:1, 3200:3200+O], rr(cur1[0:1, 3232:3232+O]), rr(cur1[0:1, 3264:3264+O])
            b2s, b2h, b2l = cur2[0:1, 0:H], rr(cur2[0:1, 1024:1024+H]), rr(cur2[0:1, 2048:2048+H])
            ones_r = rr(cur2[0:1, 3072:3072+NSLOT*B])             # fp32r ones

            nc.sync.dma_start(out=eye[:], in_=eye_d[:])

            # ================= SETUP =================
            with (
                tc.tile_pool(name="setup_sb", bufs=1) as sp,
                tc.tile_pool(name="setup_ps", bufs=4, space="PSUM") as spp,
            ):
                # bias splits (hi/lo in fp32r)
                def bias_split(b_dram, n, bs, bh, bl):
                    nc.sync.dma_start(out=bs, in_=b_dram[:].rearrange("(a n) -> a n", a=1))
                    nc.vector.tensor_copy(out=bh, in_=bs)
                    nc.vector.tensor_sub(bl, bs, rd(bh))

                bias_split(b1_d, H, b1s, b1h, b1l)
                bias_split(b2_d, H, b2s, b2h, b2l)
                bias_split(b3_d, HL, b3s, b3h, b3l)
                bias_split(b4_d, O, b4s, b4h, b4l)

                # zero-init all weight tiles (padding regions stay 0)
                for tl in [w1h, w1l, w1b4, w3h, w3l, w4h, w4l] + w2h + w2l:
                    nc.vector.memset(rd(tl[:]), 0.0)

                def split_sb(src_ap, hi_ap, lo_ap):
                    """fp32r hi/lo split of an SBUF source without touching
                    the DVE or PE (Act copy rounds; GpSimd subtracts)."""
                    nc.scalar.copy(out=hi_ap, in_=src_ap)
                    nc.gpsimd.tensor_sub(lo_ap, src_ap, rd(hi_ap))

                # ---- W2 (host-pretransposed: w2t[k, m]) ----
                for c in range(8):
                    kw = 128 if c < 7 else H - 7 * 128
                    ws = sp.tile([128, H], F32, tag="w2s", bufs=2)
                    nc.sync.dma_start(
                        out=ws[:kw, :], in_=W2_d[c * 128 : c * 128 + kw, :]
                    )
                    split_sb(ws[:kw, :], w2h[c][:kw, 0:H], w2l[c][:kw, 0:H])
                # bias rows: k-chunk 7, partition 104 (feature 1000)
                nc.sync.dma_start(out=w2h[7][104:105, 0:H], in_=b2h)
                nc.sync.dma_start(out=w2l[7][104:105, 0:H], in_=b2l)

                # ---- W1 (w1t[f, m]; feature 128 into the stacked bias) ----
                ws1 = sp.tile([128, H], F32, tag="w1s")
                nc.sync.dma_start(out=ws1[:], in_=W1_d[0:128, :])
                split_sb(ws1[:], w1h[:, 0:H], w1l[:, 0:H])
                # feature-128 row: split at base 0, then DMA the lo part
                # to its base-32 home (F32R->F32R DMA keeps the verifier's
                # rounded-provenance happy)
                wb = cur1[0:1, 0:H]
                nc.sync.dma_start(out=wb, in_=W1_d[128:129, :])
                lo_tmp = rr(cur2[0:1, 0:H])
                nc.scalar.copy(out=w1b4[0:1, 0:H], in_=wb)
                nc.gpsimd.tensor_sub(lo_tmp, wb, rd(w1b4[0:1, 0:H]))
                nc.sync.dma_start(out=w1b4[32:33, 0:H], in_=lo_tmp)
                nc.sync.dma_start(out=w1b4[1:2, 0:H], in_=b1h)
                nc.sync.dma_start(out=w1b4[33:34, 0:H], in_=b1l)

                # ---- W3 (w3t[k, 20]) ----
                for c in range(8):
                    kw = 128 if c < 7 else H - 7 * 128
                    ws3 = sp.tile([128, HL], F32, tag="w3s", bufs=2)
                    nc.sync.dma_start(
                        out=ws3[:kw, :], in_=W3_d[c * 128 : c * 128 + kw, :]
                    )
                    split_sb(
                        ws3[:kw, :],
                        w3h[:kw, c * HL : (c + 1) * HL],
                        w3l[:kw, c * HL : (c + 1) * HL],
                    )
                nc.sync.dma_start(out=w3h[104:105, 7 * HL : 8 * HL], in_=b3h)
                nc.sync.dma_start(out=w3l[104:105, 7 * HL : 8 * HL], in_=b3l)

                # ---- W4 (w4t[20, 10]) ----
                ws4 = cur2[0:HL, 1024:1024+O]
                nc.sync.dma_start(out=ws4, in_=W4_d[:])
                split_sb(ws4, w4h[:HL, :], w4l[:HL, :])
                nc.sync.dma_start(out=w4h[HL : HL + 1, :], in_=b4h)
                nc.sync.dma_start(out=w4l[HL : HL + 1, :], in_=b4l)

                # ---- state init ----
                for tl in [mem1, mem2, mem3]:
                    nc.vector.memset(tl[:], 0.0)
                nc.vector.memset(spk4[:], 0.0)
                nc.vector.memset(mem4x[0:10, :], 0.0)
                nc.vector.memset(rd(spk1[:]), 0.0)
                nc.vector.memset(rd(spk2[:]), 0.0)
                nc.vector.memset(rd(spk3[:]), 0.0)
                # constant-one bias rhs rows (all ring slots).  Compute
                # engines need 32-aligned partition starts, so write these
                # single rows via DMA from the fp32r ones region.
                nc.vector.memset(rd(ones_r), 1.0)
                s1w = spk1[:].rearrange("p (s c b) -> p s c b", s=NSLOT, b=B)
                s2w = spk2[:].rearrange("p (s c b) -> p s c b", s=NSLOT, b=B)
                ones_v = ones_r.rearrange("p (s b) -> p s b", s=NSLOT)
                nc.sync.dma_start(out=s1w[104:105, :, 7, :], in_=ones_v)
                nc.sync.dma_start(out=s2w[104:105, :, 7, :], in_=ones_v)
                # forever-spike driver for padding neurons
                big_r = rr(cur1[0:1, 3328:3328+24])
                nc.vector.memset(rd(big_r), 64.0)
                nc.sync.dma_start(out=w1b4[1:2, H:HP], in_=big_r)
                nc.sync.dma_start(out=w2h[7][104:105, H:HP], in_=big_r)
                nc.sync.dma_start(
                    out=spk3[HL : HL + 1, :],
                    in_=ones_r.rearrange("p (s b) -> p s b", s=2)[:, 0, :],
                )
                # f32 ones row (1.0 has identical f32r/f32 bits)
                nc.sync.dma_start(
                    out=mem4x[10:11, :],
                    in_=rd(ones_r).rearrange("p (s b) -> p s b", s=2)[:, 0, :],
                )
                # clear the scratch regions before the scan
                nc.vector.memset(cur1[:], 0.0)
                nc.vector.memset(cur2[:], 0.0)

            # ================= SCAN =================
            with (
                tc.tile_pool(name="xdma", bufs=2) as xdp,
                tc.tile_pool(name="xbdma", bufs=1) as xbp,
                tc.tile_pool(name="xsplit", bufs=1) as xsp,
                tc.tile_pool(name="pl1", bufs=3, space="PSUM") as pl1,
                tc.tile_pool(name="pl2", bufs=2, space="PSUM") as pl2,
                tc.tile_pool(name="pl34", bufs=1, space="PSUM") as pl34,
                tc.tile_pool(name="pout", bufs=1, space="PSUM") as pout,
            ):
                evac = {"done": 0, "tile": None}

                xv = x_d[:].rearrange("f (t b) -> f t b", b=B)
                s1_v = spk1[:].rearrange("p (s c b) -> p s c b", s=NSLOT, b=B)
                s2_v = spk2[:].rearrange("p (s c b) -> p s c b", s=NSLOT, b=B)
                m1_v = mem1[:].rearrange("p (c b) -> p c b", b=B)
                m2_v = mem2[:].rearrange("p (c b) -> p c b", b=B)
                c1_v = cur1[:].rearrange("p (s c b) -> p s c b", s=Q, b=B)
                c2_v = cur2[:].rearrange("p (s c b) -> p s c b", s=Q, b=B)
                s3_v = spk3[:].rearrange("p (s b) -> p s b", s=Q)
                mem4 = mem4x[0:O, :]
                s4_v = spk4[:].rearrange("p (s b) -> p s b", s=Q)
                m4_v = mem4.rearrange("p (s b) -> p s b", s=Q)

                def split_x(q):
                    """DMA the x slice for quad q and make fp32r hi/lo
                    splits (exact for x).  Steps past T keep stale (finite)
                    ring data; they are never read back meaningfully."""
                    t0 = Q * q
                    nt = min(Q, T - t0)
                    xq = xdp.tile([128, Q * B], F32, tag="xq")
                    xbr = xbp.tile([2, Q * B], F32, tag="xbr")
                    xq_w = xq[:].rearrange("p (t b) -> p t b", b=B)
                    xbr_w = xbr[:].rearrange("p (t b) -> p t b", b=B)
                    nc.sync.dma_start(
                        out=xq_w[:, 0:nt, :], in_=xv[0:128, t0 : t0 + nt, :]
                    )
                    nc.sync.dma_start(
                        out=xbr_w[0:1, 0:nt, :], in_=xv[128:129, t0 : t0 + nt, :]
                    )
                    if q < 1:
                        # single buffer: write the ones row once
                        nc.sync.dma_start(out=xbr[1:2, :], in_=mem4x[10:11, :])
                    xh = xsp.tile([128, Q * B], F32R, tag="xh")
                    xl = xsp.tile([128, Q * B], F32R, tag="xl")
                    xbl = xsp.tile([2, Q * B], F32R, tag="xbl")
                    # stacked moving operand for the L1 bias terms: rows 0-1
                    # and 32-33 both carry {feat128-hi, ones}; rows 2-31 stay
                    # zero from the pre-loop memset (bufs=1: same memory).
                    xb34 = xsp.tile([34, Q * B], F32R, tag="xb34")
                    nc.scalar.copy(out=xh[:], in_=xq[:])
                    nc.gpsimd.tensor_sub(xl[:], xq[:], rd(xh[:]))
                    nc.scalar.copy(out=xb34[0:2, :], in_=xbr[:])
                    nc.scalar.copy(out=xb34[32:34, :], in_=xbr[:])
                    nc.gpsimd.tensor_sub(xbl[:], xbr[:], rd(xb34[0:2, :]))
                    return xh, xl, xbl, xb34

                def mm1(q, xs):
                    """L1 matmuls for quad q; evac each m-chunk to cur1."""
                    xh, xl, xbl, xb34 = xs
                    for mc in range(8):
                        t1 = pl1.tile([128, Q * B], F32, tag="l1")
                        ms = slice(mc * 128, (mc + 1) * 128)
                        terms = [
                            (w1h[:, ms], xh[:]),
                            (w1b4[:, ms], xb34[:]),
                            (w1h[:, ms], xl[:]),
                            (w1b4[0:2, ms], xbl[:]),
                            (w1l[:, ms], xh[:]),
                        ]
                        for i, (lhsT, rhs) in enumerate(terms):
                            nc.tensor.matmul(
                                t1[:], lhsT=lhsT, rhs=rhs,
                                start=(i == 0), stop=(i == len(terms) - 1),
                            )
                        nc.scalar.copy(out=c1_v[:, :, mc, :], in_=t1[:].rearrange("p (s b) -> p s b", b=B))

                def mm2(q):
                    """L2 matmuls for quad q; evac each m-chunk to cur2."""
                    sl0 = (Q * q) % NSLOT
                    for mc in range(8):
                        t2 = pl2.tile([128, Q * B], F32, tag="l2")
                        n = 0
                        for wsp in (w2h, w2l):
                            for c in range(8):
                                nc.tensor.matmul(
                                    t2[:],
                                    lhsT=wsp[c][:, mc * 128 : (mc + 1) * 128],
                                    rhs=s1_v[:, sl0 : sl0 + Q, c, :],
                                    start=(n == 0), stop=(n == 15),
                                )
                                n += 1
                        nc.scalar.copy(out=c2_v[:, :, mc, :], in_=t2[:].rearrange("p (s b) -> p s b", b=B))

                def lif12(q, m_v, c_v, s_v, mem, spk, eng):
                    t0 = Q * q
                    for s in range(Q):
                        t = t0 + s
                        if t >= T:
                            break
                        sl, slp = t % NSLOT, (t - 1) % NSLOT
                        eng.scalar_tensor_tensor(
                            out=m_v[:, :, :], in0=m_v[:, :, :], scalar=BETA,
                            in1=c_v[:, s, :, :], op0=MULT, op1=ADD,
                        )
                        nc.gpsimd.tensor_sub(
                            mem[:], mem[:],
                            rd(spk[:, slp * HP : (slp + 1) * HP]),
                        )
                        eng.tensor_scalar(
                            out=s_v[:, sl, 0:7, :], in0=m_v[:, 0:7, :],
                            scalar1=TH, scalar2=None, op0=GT,
                        )
                        eng.tensor_scalar(
                            out=s_v[0:104, sl, 7, :], in0=m_v[0:104, 7, :],
                            scalar1=TH, scalar2=None, op0=GT,
                        )

                def mm3(q):
                    sl0 = (Q * q) % NSLOT
                    t3 = pl34.tile([32, Q * B], F32, tag="l3")
                    n = 0
                    for wsp in (w3h, w3l):
                        for c in range(8):
                            nc.tensor.matmul(
                                t3[0:HL, :],
                                lhsT=wsp[:, c * HL : (c + 1) * HL],
                                rhs=s2_v[:, sl0 : sl0 + Q, c, :],
                                start=(n == 0), stop=(n == 15),
                            )
                            n += 1
                    return t3

                def lif3(q, t3):
                    t0 = Q * q
                    for s in range(Q):
                        t = t0 + s
                        if t >= T:
                            break
                        sl, slp = t % Q, (t - 1) % Q
                        nc.vector.scalar_tensor_tensor(
                            out=mem3[:], in0=mem3[:], scalar=BETA,
                            in1=t3[0:HL, s * B : (s + 1) * B], op0=MULT, op1=ADD,
                        )
                        nc.gpsimd.tensor_sub(
                            mem3[:], mem3[:], rd(s3_v[0:HL, slp, :])
                        )
                        nc.vector.tensor_scalar(
                            out=s3_v[0:HL, sl, :], in0=mem3[:],
                            scalar1=TH, scalar2=None, op0=GT,
                        )

                def mm4(q):
                    t4 = pl34.tile([32, Q * B], F32, tag="l4")
                    nc.tensor.matmul(
                        t4[0:O, :], lhsT=w4h[:], rhs=spk3[:],
                        start=True, stop=False,
                    )
                    nc.tensor.matmul(
                        t4[0:O, :], lhsT=w4l[:], rhs=spk3[:],
                        start=False, stop=True,
                    )
                    return t4

                def lif4(q, t4):
                    t0 = Q * q
                    for s in range(Q):
                        t = t0 + s
                        if t >= T:
                            break
                        sl, slp = t % Q, (t - 1) % Q
                        nc.vector.scalar_tensor_tensor(
                            out=m4_v[:, sl, :], in0=m4_v[:, slp, :], scalar=BETA,
                            in1=t4[0:O, s * B : (s + 1) * B], op0=MULT, op1=ADD,
                        )
                        nc.gpsimd.tensor_sub(
                            m4_v[:, sl, :], m4_v[:, sl, :], s4_v[:, slp, :]
                        )
                        nc.vector.tensor_scalar(
                            out=s4_v[:, sl, :], in0=m4_v[:, sl, :],
                            scalar1=TH, scalar2=None, op0=GT,
                        )

                def flush_block(t_end):
                    """bounce the finished 12-step block PSUM->SBUF on the
                    scalar engine, then DMA it to DRAM."""
                    d0 = evac["done"]
                    n = t_end - d0
                    blk = evac["tile"]
                    nc.scalar.copy(out=obuf[:, 0 : n * 2 * O], in_=blk[:, 0 : n * 2 * O])
                    ob = obuf[:, 0 : n * 2 * O].rearrange("b (t x) -> b t x", x=2 * O)
                    nc.sync.dma_start(
                        out=spk_o[d0:t_end].rearrange("t b o -> b t o"),
                        in_=ob[:, :, 0:O],
                    )
                    nc.sync.dma_start(
                        out=mem_o[d0:t_end].rearrange("t b o -> b t o"),
                        in_=ob[:, :, O : 2 * O],
                    )
                    evac["done"] = t_end
                    evac["tile"] = None

                def record(q):
                    t0 = Q * q
                    if evac["tile"] is None:
                        evac["tile"] = pout.tile(
                            [128, 8 * 2 * O], F32, tag="outacc", name="outacc"
                        )
                    outacc = evac["tile"]
                    for s in range(Q):
                        t = t0 + s
                        if t >= T:
                            break
                        sl = t % Q
                        w = t - evac["done"]
                        nc.tensor.transpose(
                            outacc[:, w * 2 * O : w * 2 * O + O],
                            spk4[:, sl * B : (sl + 1) * B],
                            eye[:O, :O],
                        )
                        nc.tensor.transpose(
                            outacc[:, w * 2 * O + O : (w + 1) * 2 * O],
                            mem4[:, sl * B : (sl + 1) * B],
                            eye[:O, :O],
                        )
                    t_end = min(t0 + Q, T)
                    if t_end - evac["done"] == 8 or t_end == T:
                        flush_block(t_end)

                # ---- prologue: zero the stacked-xb pool slot once, then
                # x splits for quads 0/1, L1+lif1(0) ----
                xb34_init = xsp.tile([34, Q * B], F32R, tag="xb34", name="xb34_init")
                nc.vector.memset(rd(xb34_init[:]), 0.0)
                xs = {0: split_x(0), 1: split_x(1)}
                mm1(0, xs[0])
                lif12(0, m1_v, c1_v, s1_v, mem1, spk1, nc.vector)

                # ---- main loop: PE never waits ----
                # PE:   mm1(q+1) | mm2(q) | mm3(q-1) mm4(q-1) rec(q-1)
                # DVE:  lif1(q+1) | x-lo sub(q+2) | lif2(q)
                # Pool: lif3(q-1) lif4(q-1)
                for q in range(NQ):
                    if q + 1 < NQ:
                        mm1(q + 1, xs.pop(q + 1))
                    mm2(q)
                    if q + 1 < NQ:
                        lif12(q + 1, m1_v, c1_v, s1_v, mem1, spk1, nc.vector)
                    if q + 2 < NQ:
                        xs[q + 2] = split_x(q + 2)
                    if q > 0:
                        t3p = mm3(q - 1)
                        lif3(q - 1, t3p)
                        t4p = mm4(q - 1)
                        lif4(q - 1, t4p)
                        record(q - 1)
                    lif12(q, m2_v, c2_v, s2_v, mem2, spk2, nc.vector)

                # ---- epilogue: L3/L4 for the last quad ----
                t3p = mm3(NQ - 1)
                lif3(NQ - 1, t3p)
                t4p = mm4(NQ - 1)
                lif4(NQ - 1, t4p)
                record(NQ - 1)

    fix_multi_waits(nc)
    return nc


_NC_CACHE = {}


def _get_nc(T=T_FULL):
    if T not in _NC_CACHE:
        _NC_CACHE[T] = build_nc(T)
    return _NC_CACHE[T]


def run_cores(inputs, T=T_FULL, n_cores=NCORES, **kw):
    """Run on the first n_cores with batch n_cores*128; returns (spk, mem)."""
    nc = _get_nc(T)
    eye = np.eye(128, dtype=np.float32)
    base = {k: np.asarray(inputs[k], np.float32)
            for k in ("b1", "b2", "b3", "b4")}
    base["eye"] = eye
    for nm in ("W1", "W2", "W3", "W4"):
        base[nm.lower() + "t"] = np.ascontiguousarray(
            np.asarray(inputs[nm], np.float32).T
        )
    x = np.asarray(inputs["x"], np.float32)
    in_maps = []
    for c in range(n_cores):
        xc = x[c * B : (c + 1) * B].reshape(B, F, T_FULL)
        xt = np.ascontiguousarray(xc.transpose(1, 2, 0)).reshape(F, T_FULL * B)
        in_maps.append(dict(base, xt=xt))
    res = run_bass_kernel_spmd(nc, in_maps, list(range(n_cores)), **kw)
    run_cores.last_result = res
    spk = np.concatenate([res.results[c]["spk_out"] for c in range(n_cores)], axis=1)
    mem = np.concatenate([res.results[c]["mem_out"] for c in range(n_cores)], axis=1)
    return spk, mem


def kernel(x, W1, b1, W2, b2, W3, b3, W4, b4):
    spk, mem = run_cores(
        dict(x=x, W1=W1, b1=b1, W2=W2, b2=b2, W3=W3, b3=b3, W4=W4, b4=b4)
    )
    return spk, mem
